# revision 1
# baseline (speedup 1.0000x reference)
"""Multi-head cosine self-attention on 8 Trainium2 NeuronCores (Bass/Tile).

Problem: y = MHA(x) with L2-normalized q/k (cosine attention) and per-head
scaling sim / n**sigmoid(m);  x: [4, 2048, 1024], 16 heads of dim 64.

Sharding: core c handles batch c//2 and head-group c%2 (8 heads = 512 of the
1024 q/k/v features).  Each core computes its partial output
(attn_out_part @ Wo[rows]); the host sums the two partials per batch and adds
bo.  No collectives.

Per-core layout strategy (everything transposed, f' on partitions):
  - host passes x[b].T, so xT streams straight into SBUF
  - qT/kT = W.T @ xT via PE (f' on partitions, 2 heads per 128-partition tile)
  - row norms of q/k = matmul(ones_block, qT*qT) -> [2, n] per head pair
  - 1/(||q||*n^sig) broadcast to 128 partitions via a K=2 indicator matmul,
    applied in-place to qT/kT (DVE) so sim needs no further scaling
  - simT[j,i] = khatT.T @ qhatT per head with K=64 row-packing (2 heads
    concurrently in PE rows 0-63 / 64-127)
  - out2T[d,i] = sum_j v[j,d] * attnT[j,i] with M=64 col-packing (2 heads in
    PE cols 0-63 / 64-127 of one PSUM bank)
  - attn_outT kept in SBUF (bf16), final projection = attn_outT.T @ Wo_rows
Matmuls use float32r (full PE rate at N=512, fp32 storage) except the
attn@V stage and final projection, which run bf16 (col-tiling + eviction
bandwidth).  Phase-1 loops run head-pair-outer so each pair's attention
overlaps the remaining projections; sim PSUMs are paired into 2-bank
[128,1024] tiles to halve eviction instruction count.
"""

import os
import sys

for _p in ("/opt/trn_rl_repo",):
    if os.path.isdir(_p) and _p not in sys.path:
        sys.path.insert(0, _p)

from contextlib import ExitStack

import ml_dtypes
import numpy as np

import concourse.bacc as bacc
import concourse.mybir as mybir
import concourse.tile as tile
from concourse import bass_utils

P = 128
F = 1024  # model dim
H = 16  # total heads
HD = 64  # head dim
G = 2  # head groups (tensor-parallel factor)
FG = F // G  # 512 features per core
PAIRS = FG // P  # 4 head-pairs per core
KT = F // P  # 8 contraction tiles for the projections
NCORES = 8
F32 = mybir.dt.float32
FR = mybir.dt.float32r
BF = mybir.dt.bfloat16
AF = mybir.ActivationFunctionType


def _mm(nc, out, lhsT, rhs, **kw):
    nc.tensor.matmul(out, lhsT, rhs, **kw)


def build_core_program(nc, n=2048):
    NC = n // 512  # i-chunks
    NT = n // P  # n-tiles (= j-tiles)
    NTC = 512 // P  # n-tiles per i-chunk

    xt = nc.dram_tensor("xt", [P, NC, KT, 512], FR, kind="ExternalInput").ap()
    wq = nc.dram_tensor("wq", [P, PAIRS, KT, P], FR, kind="ExternalInput").ap()
    wk = nc.dram_tensor("wk", [P, PAIRS, KT, P], FR, kind="ExternalInput").ap()
    wv = nc.dram_tensor("wv", [P, KT, FG], FR, kind="ExternalInput").ap()
    wo = nc.dram_tensor("wo", [P, PAIRS, F], BF, kind="ExternalInput").ap()
    bqd = nc.dram_tensor("bq", [P, PAIRS], F32, kind="ExternalInput").ap()
    bkd = nc.dram_tensor("bk", [P, PAIRS], F32, kind="ExternalInput").ap()
    bvd = nc.dram_tensor("bv", [FG], FR, kind="ExternalInput").ap()
    # cmsq[a, p] = (n ** sigmoid(m))**2 for local head 2p+a
    cmsq = nc.dram_tensor("cmsq", [2, PAIRS], F32, kind="ExternalInput").ap()
    cind = nc.dram_tensor("cind", [2, P], FR, kind="ExternalInput").ap()
    cblk = nc.dram_tensor("cblk", [P, 2], FR, kind="ExternalInput").ap()
    cones = nc.dram_tensor("cones", [1, P], FR, kind="ExternalInput").ap()
    out = nc.dram_tensor("out", [n, F], F32, kind="ExternalOutput").ap()

    with tile.TileContext(nc) as tc, ExitStack() as ctx:
        const = ctx.enter_context(tc.tile_pool(name="const", bufs=1))
        persist = ctx.enter_context(tc.tile_pool(name="persist", bufs=1))
        ps = ctx.enter_context(tc.tile_pool(name="ps", bufs=1, space="PSUM"))

        # --- constants ---------------------------------------------------
        ones_blk = const.tile([P, 2], FR)  # block col-sums for head-pair norms
        nc.sync.dma_start(ones_blk[:], cblk)
        ind = const.tile([2, P], FR)  # partition-broadcast indicator
        nc.sync.dma_start(ind[:], cind)
        ones_row = const.tile([1, P], FR)  # bias outer-product row
        nc.sync.dma_start(ones_row[:], cones)
        zcol = const.tile([P, 1], F32)  # explicit zero bias for ACT
        nc.any.memset(zcol[:], 0.0)

        bq_sb = const.tile([P, PAIRS], F32)
        nc.sync.dma_start(bq_sb[:], bqd)
        bk_sb = const.tile([P, PAIRS], F32)
        nc.sync.dma_start(bk_sb[:], bkd)
        bv_sb = const.tile([1, FG], FR)
        nc.sync.dma_start(bv_sb[:], bvd[None, :])
        cm_sb = const.tile([2, PAIRS], F32)
        nc.sync.dma_start(cm_sb[:], cmsq)

        # --- persistent activations -------------------------------------
        qT = persist.tile([P, PAIRS, n], FR)  # (x Wq + bq)^T, 2 heads/tile
        kT = persist.tile([P, PAIRS, n], FR)
        v = persist.tile([P, NT, FG], BF)  # x Wv + bv, natural layout

        # ================= phase 1: q/k/v projections ====================
        # v first, then q/k with the head-pair (ft) loop OUTER so pair ft's
        # attention in phase 2 unblocks as soon as its qT/kT slice is done.
        with tc.tile_pool(name="ph1", bufs=1) as ph1:
            xall = ph1.tile([P, NC, KT, 512], FR)
            for ic in range(NC):
                nc.sync.dma_start(xall[:, ic], xt[:, ic])
            wv_sb = ph1.tile([P, KT, FG], FR)
            nc.sync.dma_start(wv_sb[:], wv)
            for ic in range(NC):
                for jt in range(NTC):
                    nt_idx = ic * NTC + jt
                    jsl = slice(jt * P, (jt + 1) * P)
                    pt = ps.tile([P, FG], F32, tag="mm", bufs=2)
                    for k in range(KT):
                        _mm(nc, pt, xall[:, ic, k, jsl], wv_sb[:, k, :],
                            start=(k == 0), stop=False)
                    # + 1s^T bv outer product adds the bias to every row
                    _mm(nc, pt, ones_row, bv_sb, start=False, stop=True)
                    nc.scalar.activation(v[:, nt_idx, :], pt, AF.Identity,
                                         bias=zcol[:])
            for ft in range(PAIRS):
                for wdr, bsb, dstT, wtag in ((wq, bq_sb, qT, "wqf"),
                                             (wk, bk_sb, kT, "wkf")):
                    wf = ph1.tile([P, KT, P], FR, tag=wtag, bufs=2)
                    nc.sync.dma_start(wf[:], wdr[:, ft])
                    for ic in range(NC):
                        isl = slice(ic * 512, (ic + 1) * 512)
                        pt = ps.tile([P, 512], F32, tag="mm", bufs=2)
                        for k in range(KT):
                            _mm(nc, pt, wf[:, k, :], xall[:, ic, k, :],
                                start=(k == 0), stop=(k == KT - 1))
                        nc.scalar.activation(dstT[:, ft, isl], pt, AF.Identity,
                                             bias=bsb[:, ft:ft + 1])

        # ================= phase 2: cosine attention =====================
        with tc.tile_pool(name="ph2", bufs=1) as ph2:
            aoT = ph2.tile([P, PAIRS, n], BF, tag="aoT")  # attn-out^T
            for pr in range(PAIRS):
                # ---- 1/(||q|| * n^sig) and 1/||k|| as [2, n] rows -------
                for src, scale_ap in ((qT, cm_sb[:, pr:pr + 1]),
                                      (kT, None)):
                    row = ph2.tile([2, n], F32, tag="row", bufs=2)
                    sq = ph2.tile([P, n], FR, tag="sq", bufs=2)
                    nc.scalar.activation(sq[:], src[:, pr, :], AF.Square,
                                         bias=zcol[:])
                    for ch in range(NC):
                        csl = slice(ch * 512, (ch + 1) * 512)
                        nps = ps.tile([2, 512], F32, tag="mm", bufs=2)
                        _mm(nc, nps, ones_blk, sq[:, csl], start=True, stop=True)
                        if scale_ap is not None:
                            nc.scalar.activation(row[:, csl], nps, AF.Sqrt,
                                                 bias=zcol[:2], scale=scale_ap)
                        else:
                            nc.scalar.activation(row[:, csl], nps, AF.Sqrt,
                                                 bias=zcol[:2])
                    nc.vector.reciprocal(row[:], row[:])
                    rowr = ph2.tile([2, n], FR, tag="rowr", bufs=2)
                    nc.vector.tensor_copy(rowr[:], row[:])
                    # broadcast row across partitions and apply in place
                    for ch in range(NC):
                        csl = slice(ch * 512, (ch + 1) * 512)
                        bps = ps.tile([P, 512], F32, tag="mm", bufs=2)
                        _mm(nc, bps, ind, rowr[:, csl], start=True, stop=True)
                        nc.vector.tensor_tensor(src[:, pr, csl], src[:, pr, csl],
                                                bps, mybir.AluOpType.mult)

                # ---- simT -> attnT -> out2T per i-chunk -----------------
                for ic in range(NC):
                    isl = slice(ic * 512, (ic + 1) * 512)
                    avp = ps.tile([P, 512], F32, tag="av", bufs=2)
                    for j in range(NT):
                        jsl = slice(j * P, (j + 1) * P)
                        sp2 = ps.tile([P, 1024], F32, tag="mm2", bufs=2)
                        for po in (0, HD):  # head 2pr (rows 0-63), 2pr+1
                            _mm(nc, sp2[:, 8 * po:8 * po + 512],
                                kT[po:po + HD, pr, jsl],
                                qT[po:po + HD, pr, isl],
                                start=True, stop=True, tile_position=(po, 0))
                        at = ph2.tile([P, 1024], BF, tag="at", bufs=6)
                        if j % 2 == 0:
                            nc.vector.tensor_copy(at[:], sp2)
                        else:
                            nc.scalar.copy(at[:], sp2)
                        for po in (0, HD):
                            _mm(nc, avp[po:po + HD, :],
                                v[:, j, pr * P + po:pr * P + po + HD],
                                at[:, 8 * po:8 * po + 512],
                                start=(j == 0), stop=(j == NT - 1),
                                tile_position=(0, po), skip_group_check=True)
                    nc.vector.tensor_copy(aoT[:, pr, isl], avp)

            # ================= phase 3: output projection ================
            wo_sb = ph2.tile([P, PAIRS, F], BF, tag="wo")
            nc.sync.dma_start(wo_sb[:], wo)
            for nt in range(NT):
                ntsl = slice(nt * P, (nt + 1) * P)
                pt2 = ps.tile([P, F], F32, tag="mm2", bufs=2)
                for fc in range(F // 512):
                    fsl = slice(fc * 512, (fc + 1) * 512)
                    for kt in range(PAIRS):
                        _mm(nc, pt2[:, fsl], aoT[:, kt, ntsl], wo_sb[:, kt, fsl],
                            start=(kt == 0), stop=(kt == PAIRS - 1))
                ost = ph2.tile([P, F], F32, tag="ost", bufs=2)
                nc.vector.tensor_copy(ost[:], pt2)
                nc.sync.dma_start(out[ntsl, :], ost[:])
    return nc


_CACHE = {}


def get_nc(n=2048):
    if n not in _CACHE:
        nc = bacc.Bacc("TRN2", target_bir_lowering=False, debug=False,
                       num_devices=NCORES)
        build_core_program(nc, n)
        nc.compile()
        _CACHE[n] = nc
    return _CACHE[n]


def _warr(W, sl):
    return np.ascontiguousarray(
        np.asarray(W, np.float32)[:, sl].reshape(KT, P, FG).transpose(1, 0, 2))


def _warr_ft(W, sl):
    return np.ascontiguousarray(
        np.asarray(W, np.float32)[:, sl].reshape(KT, P, PAIRS, P)
        .transpose(1, 2, 0, 3))


_IND = np.zeros((2, P), np.float32)
_IND[0, :HD] = 1.0
_IND[1, HD:] = 1.0
_BLK = np.zeros((P, 2), np.float32)
_BLK[:HD, 0] = 1.0
_BLK[HD:, 1] = 1.0
_ONES = np.ones((1, P), np.float32)


def make_in_maps(x, Wq, bq, Wk, bk, Wv, bv, Wo, bo, m):
    n = x.shape[1]
    sig = 1.0 / (1.0 + np.exp(-np.asarray(m, np.float64)))
    scale = np.float64(n) ** sig  # [16] per-head n^sigmoid(m)
    in_maps = []
    for c in range(NCORES):
        bi, g = divmod(c, 2)
        sl = slice(g * FG, (g + 1) * FG)
        hsc = scale[g * (H // G):(g + 1) * (H // G)]  # 8 local heads
        cm = (hsc ** 2).reshape(PAIRS, 2).T  # [2, PAIRS]
        xa = np.asarray(x[bi], np.float32)
        NCc = n // 512
        in_maps.append({
            "xt": np.ascontiguousarray(
                xa.reshape(NCc, 512, KT, P).transpose(3, 0, 2, 1)),
            "wq": _warr_ft(Wq, sl), "wk": _warr_ft(Wk, sl), "wv": _warr(Wv, sl),
            "wo": np.ascontiguousarray(
                np.asarray(Wo, np.float32)[sl].reshape(PAIRS, P, F)
                .transpose(1, 0, 2).astype(ml_dtypes.bfloat16)),
            "bq": np.ascontiguousarray(np.asarray(bq, np.float32)[sl].reshape(PAIRS, P).T),
            "bk": np.ascontiguousarray(np.asarray(bk, np.float32)[sl].reshape(PAIRS, P).T),
            "bv": np.ascontiguousarray(np.asarray(bv, np.float32)[sl]),
            "cmsq": np.ascontiguousarray(cm.astype(np.float32)),
            "cind": _IND,
            "cblk": _BLK,
            "cones": _ONES,
        })
    return in_maps


def kernel(x, Wq, bq, Wk, bk, Wv, bv, Wo, bo, m, _trace=False):
    x = np.asarray(x, np.float32)
    b, n, f = x.shape
    nc = get_nc(n)
    in_maps = make_in_maps(x, Wq, bq, Wk, bk, Wv, bv, Wo, bo, m)
    res = bass_utils.run_bass_kernel_spmd(nc, in_maps,
                                          core_ids=list(range(NCORES)),
                                          trace=_trace)
    outs = [r["out"] for r in res.results]
    y = np.empty((b, n, f), np.float32)
    for bi in range(b):
        y[bi] = outs[2 * bi] + outs[2 * bi + 1]
    y += np.asarray(bo, np.float32).reshape(1, 1, f)
    if _trace:
        kernel._last_results = res
    return y


if __name__ == "__main__":
    # build-only smoke test (no device)
    nc = bacc.Bacc("TRN2", target_bir_lowering=False, debug=False,
                   num_devices=NCORES)
    build_core_program(nc, n=int(sys.argv[1]) if len(sys.argv) > 1 else 2048)
    print("build OK:", len(nc.m.functions[0].blocks[0].instructions) if nc.m.functions else "?", "instructions-ish")



# revision 4
# speedup vs baseline: 206.2318x; 206.2318x over previous
"""Multi-head cosine self-attention on 8 Trainium2 NeuronCores (Bass/Tile).

Problem: y = MHA(x) with L2-normalized q/k (cosine attention) and per-head
scaling sim / n**sigmoid(m);  x: [4, 2048, 1024], 16 heads of dim 64.

Sharding: core c handles batch c//2 and head-group c%2 (8 heads = 512 of the
1024 q/k/v features).  Each core computes its partial output
(attn_out_part @ Wo[rows]); the host sums the two partials per batch and adds
bo.  No collectives.

Per-core layout strategy (everything transposed, f' on partitions, bf16):
  - host passes x[b].T in bf16, so xT streams straight into SBUF
  - qT/kT = W.T @ xT via PE (f' on partitions, 2 heads per 128-partition tile)
  - row norms of q/k = matmul(ones_block, qT*qT) -> [2, n] per head pair;
    1/(||q||*n^sig) via one ACT Rsqrt (scale folds the n^sig factor)
  - the [2, n] factor row is broadcast to 128 partitions via a K=2 indicator
    matmul and applied in-place to qT/kT (DVE) so sim needs no further scaling
  - simT[j,i] = khatT.T @ qhatT per head with K=64 row-packing (2 heads
    concurrently in PE rows 0-63 / 64-127)
  - out2T[d,i] = sum_j v[j,d] * attnT[j,i] with M=64 col-packing (2 heads in
    PE cols 0-63 / 64-127 of one PSUM bank)
  - attn_outT kept in SBUF (bf16), final projection = attn_outT.T @ Wo_rows
Program order interleaves pair p's attention with pair p+1's q/k projections
and norm prep, and the final pair's attention with the output projection, so
the PE always has projection work to fill eviction-bound gaps.  sim PSUMs are
paired into 2-bank [128,1024] tiles; evictions alternate DVE/ACT.
"""

import os
import sys

for _p in ("/opt/trn_rl_repo",):
    if os.path.isdir(_p) and _p not in sys.path:
        sys.path.insert(0, _p)

from contextlib import ExitStack

import ml_dtypes
import numpy as np

import concourse.bacc as bacc
import concourse.mybir as mybir
import concourse.tile as tile
from concourse import bass_utils

P = 128
F = 1024  # model dim
H = 16  # total heads
HD = 64  # head dim
G = 2  # head groups (tensor-parallel factor)
FG = F // G  # 512 features per core
PAIRS = FG // P  # 4 head-pairs per core
KT = F // P  # 8 contraction tiles for the projections
NCORES = 8
F32 = mybir.dt.float32
BF = mybir.dt.bfloat16
AF = mybir.ActivationFunctionType


def _mm(nc, out, lhsT, rhs, **kw):
    nc.tensor.matmul(out, lhsT, rhs, **kw)


def build_core_program(nc, n=2048):
    NC = n // 512  # i-chunks
    NT = n // P  # n-tiles (= j-tiles)
    NTC = 512 // P  # n-tiles per i-chunk

    xt = nc.dram_tensor("xt", [P, NC, KT, 512], BF, kind="ExternalInput").ap()
    wq = nc.dram_tensor("wq", [P, PAIRS, KT, P], BF, kind="ExternalInput").ap()
    wk = nc.dram_tensor("wk", [P, PAIRS, KT, P], BF, kind="ExternalInput").ap()
    wv = nc.dram_tensor("wv", [P, KT, FG], BF, kind="ExternalInput").ap()
    wo = nc.dram_tensor("wo", [P, PAIRS, F], BF, kind="ExternalInput").ap()
    bqd = nc.dram_tensor("bq", [P, PAIRS], F32, kind="ExternalInput").ap()
    bkd = nc.dram_tensor("bk", [P, PAIRS], F32, kind="ExternalInput").ap()
    bvd = nc.dram_tensor("bv", [FG], BF, kind="ExternalInput").ap()
    # cmsq[a, p] = (n ** sigmoid(m))**2 for local head 2p+a
    cmsq = nc.dram_tensor("cmsq", [2, PAIRS], F32, kind="ExternalInput").ap()
    cind = nc.dram_tensor("cind", [2, P], BF, kind="ExternalInput").ap()
    cblk = nc.dram_tensor("cblk", [P, 2], BF, kind="ExternalInput").ap()
    cones = nc.dram_tensor("cones", [1, P], BF, kind="ExternalInput").ap()
    out = nc.dram_tensor("out", [n, F], F32, kind="ExternalOutput").ap()

    with tile.TileContext(nc) as tc, ExitStack() as ctx:
        const = ctx.enter_context(tc.tile_pool(name="const", bufs=1))
        persist = ctx.enter_context(tc.tile_pool(name="persist", bufs=1))
        work = ctx.enter_context(tc.tile_pool(name="work", bufs=1))
        ps = ctx.enter_context(tc.tile_pool(name="ps", bufs=1, space="PSUM"))

        # --- constants ---------------------------------------------------
        ones_blk = const.tile([P, 2], BF)  # block col-sums for head-pair norms
        nc.sync.dma_start(ones_blk[:], cblk)
        ind = const.tile([2, P], BF)  # partition-broadcast indicator
        nc.sync.dma_start(ind[:], cind)
        ones_row = const.tile([1, P], BF)  # bias outer-product row
        nc.sync.dma_start(ones_row[:], cones)
        zcol = const.tile([P, 1], F32)  # explicit zero bias for ACT
        nc.any.memset(zcol[:], 0.0)

        bq_sb = const.tile([P, PAIRS], F32)
        nc.sync.dma_start(bq_sb[:], bqd)
        bk_sb = const.tile([P, PAIRS], F32)
        nc.sync.dma_start(bk_sb[:], bkd)
        bv_sb = const.tile([1, FG], BF)
        nc.sync.dma_start(bv_sb[:], bvd[None, :])
        cm_sb = const.tile([2, PAIRS], F32)
        nc.sync.dma_start(cm_sb[:], cmsq)

        # --- persistent activations -------------------------------------
        qT = persist.tile([P, PAIRS, n], BF)  # (x Wq + bq)^T, 2 heads/tile
        kT = persist.tile([P, PAIRS, n], BF)
        v = persist.tile([P, NT, FG], BF)  # x Wv + bv, natural layout
        aoT = persist.tile([P, PAIRS, n], BF)  # attn-out^T

        xall = persist.tile([P, NC, KT, 512], BF)
        for ic in range(NC):
            nc.sync.dma_start(xall[:, ic], xt[:, ic])
        wv_sb = persist.tile([P, KT, FG], BF)
        nc.sync.dma_start(wv_sb[:], wv)
        wo_sb = persist.tile([P, PAIRS, F], BF)
        nc.sync.dma_start(wo_sb[:], wo)

        def emit_qk_dma(pr):
            wfs = {}
            for wdr, wtag in ((wq, "wqf"), (wk, "wkf")):
                wf = work.tile([P, KT, P], BF, tag=wtag, bufs=2)
                nc.sync.dma_start(wf[:], wdr[:, pr])
                wfs[wtag] = wf
            return wfs

        def emit_qk_chunk(pr, wfs, ic):
            # q and k projections of pair pr for one 512-row i-chunk
            isl = slice(ic * 512, (ic + 1) * 512)
            for wtag, bsb, dstT in (("wqf", bq_sb, qT), ("wkf", bk_sb, kT)):
                wf = wfs[wtag]
                pt = ps.tile([P, 512], F32, tag="mm", bufs=2)
                for k in range(KT):
                    _mm(nc, pt, wf[:, k, :], xall[:, ic, k, :],
                        start=(k == 0), stop=(k == KT - 1))
                nc.scalar.activation(dstT[:, pr, isl], pt, AF.Identity,
                                     bias=bsb[:, pr:pr + 1])

        def emit_norm(pr):
            # 1/(||q|| * n^sig) and 1/||k|| as [2, n] bf16 rows, applied
            # in place to qT/kT so sim needs no further scaling.
            # rsqrt = sqrt(cm_inv * reciprocal(norm^2)), cm_inv = n^(-2*sig).
            for src, scale_ap in ((qT, cm_sb[:, pr:pr + 1]), (kT, None)):
                sq = work.tile([P, n], BF, tag="sq", bufs=2)
                nc.vector.tensor_tensor(sq[:], src[:, pr, :], src[:, pr, :],
                                        mybir.AluOpType.mult)
                rowr = work.tile([2, n], BF, tag="rowr", bufs=2)
                for ch in range(NC):
                    csl = slice(ch * 512, (ch + 1) * 512)
                    nps = ps.tile([2, 512], F32, tag="mm", bufs=2)
                    _mm(nc, nps, ones_blk, sq[:, csl], start=True, stop=True)
                    row = work.tile([2, 512], F32, tag="row", bufs=2)
                    nc.vector.reciprocal(row[:], nps)
                    if scale_ap is not None:
                        nc.scalar.activation(rowr[:, csl], row, AF.Sqrt,
                                             bias=zcol[:2], scale=scale_ap)
                    else:
                        nc.scalar.activation(rowr[:, csl], row, AF.Sqrt,
                                             bias=zcol[:2])
                # broadcast row across partitions and apply in place
                for ch in range(NC):
                    csl = slice(ch * 512, (ch + 1) * 512)
                    bps = ps.tile([P, 512], F32, tag="mm", bufs=2)
                    _mm(nc, bps, ind, rowr[:, csl], start=True, stop=True)
                    nc.vector.tensor_tensor(src[:, pr, csl], src[:, pr, csl],
                                            bps, mybir.AluOpType.mult)

        def emit_attn_chunk(pr, ic):
            # simT -> attnT -> out2T for one 512-col i-chunk of pair pr
            isl = slice(ic * 512, (ic + 1) * 512)
            avp = ps.tile([P, 512], F32, tag="av", bufs=2)
            for j in range(NT):
                jsl = slice(j * P, (j + 1) * P)
                sp2 = ps.tile([P, 1024], F32, tag="mm2", bufs=2)
                for po in (0, HD):  # head 2pr (rows 0-63), 2pr+1
                    _mm(nc, sp2[:, (po // HD) * 512:(po // HD) * 512 + 512],
                        kT[po:po + HD, pr, jsl],
                        qT[po:po + HD, pr, isl],
                        start=True, stop=True, tile_position=(po, 0))
                at = work.tile([P, 1024], BF, tag="at", bufs=6)
                if j % 2 == 0:
                    nc.vector.tensor_copy(at[:], sp2)
                else:
                    nc.scalar.copy(at[:], sp2)
                for po in (0, HD):
                    _mm(nc, avp[po:po + HD, :],
                        v[:, j, pr * P + po:pr * P + po + HD],
                        at[:, (po // HD) * 512:(po // HD) * 512 + 512],
                        start=(j == 0), stop=(j == NT - 1),
                        tile_position=(0, po), skip_group_check=True)
            nc.scalar.copy(aoT[:, pr, isl], avp)

        def emit_out_chunk(ic):
            # final projection for the NTC row-tiles of one i-chunk
            for t in range(NTC):
                nt = ic * NTC + t
                ntsl = slice(nt * P, (nt + 1) * P)
                ost = work.tile([P, F], F32, tag="ost", bufs=2)
                for fc in range(F // 512):
                    fsl = slice(fc * 512, (fc + 1) * 512)
                    pt2 = ps.tile([P, 512], F32, tag="mm", bufs=2)
                    for kt in range(PAIRS):
                        _mm(nc, pt2, aoT[:, kt, ntsl], wo_sb[:, kt, fsl],
                            start=(kt == 0), stop=(kt == PAIRS - 1))
                    if fc % 2 == 0:
                        nc.vector.tensor_copy(ost[:, fsl], pt2)
                    else:
                        nc.scalar.copy(ost[:, fsl], pt2)
                nc.sync.dma_start(out[ntsl, :], ost[:])

        # ================= interleaved emission ==========================
        wfs = emit_qk_dma(0)
        for ic in range(NC):
            emit_qk_chunk(0, wfs, ic)
        emit_norm(0)

        # v projections: PE filler while pair 0's norm chain runs on DVE/ACT
        for ic in range(NC):
            for jt in range(NTC):
                nt_idx = ic * NTC + jt
                jsl = slice(jt * P, (jt + 1) * P)
                pt = ps.tile([P, FG], F32, tag="mm", bufs=2)
                for k in range(KT):
                    _mm(nc, pt, xall[:, ic, k, jsl], wv_sb[:, k, :],
                        start=(k == 0), stop=False)
                # + 1s^T bv outer product adds the bias to every row
                _mm(nc, pt, ones_row, bv_sb, start=False, stop=True)
                nc.scalar.activation(v[:, nt_idx, :], pt, AF.Identity,
                                     bias=zcol[:])

        for pr in range(PAIRS):
            if pr < PAIRS - 1:
                wfs = emit_qk_dma(pr + 1)
            for ic in range(NC):
                emit_attn_chunk(pr, ic)
                if pr < PAIRS - 1:
                    emit_qk_chunk(pr + 1, wfs, ic)
                else:
                    emit_out_chunk(ic)
            if pr < PAIRS - 1:
                emit_norm(pr + 1)
    return nc


_CACHE = {}


def get_nc(n=2048):
    if n not in _CACHE:
        nc = bacc.Bacc("TRN2", target_bir_lowering=False, debug=False,
                       num_devices=NCORES)
        build_core_program(nc, n)
        nc.compile()
        _CACHE[n] = nc
    return _CACHE[n]


def _warr(W, sl):
    return np.ascontiguousarray(
        np.asarray(W, np.float32)[:, sl].reshape(KT, P, FG)
        .transpose(1, 0, 2)).astype(ml_dtypes.bfloat16)


def _warr_ft(W, sl):
    return np.ascontiguousarray(
        np.asarray(W, np.float32)[:, sl].reshape(KT, P, PAIRS, P)
        .transpose(1, 2, 0, 3)).astype(ml_dtypes.bfloat16)


_IND = np.zeros((2, P), ml_dtypes.bfloat16)
_IND[0, :HD] = 1.0
_IND[1, HD:] = 1.0
_BLK = np.zeros((P, 2), ml_dtypes.bfloat16)
_BLK[:HD, 0] = 1.0
_BLK[HD:, 1] = 1.0
_ONES = np.ones((1, P), ml_dtypes.bfloat16)


def make_in_maps(x, Wq, bq, Wk, bk, Wv, bv, Wo, bo, m):
    n = x.shape[1]
    sig = 1.0 / (1.0 + np.exp(-np.asarray(m, np.float64)))
    scale = np.float64(n) ** sig  # [16] per-head n^sigmoid(m)
    in_maps = []
    for c in range(NCORES):
        bi, g = divmod(c, 2)
        sl = slice(g * FG, (g + 1) * FG)
        hsc = scale[g * (H // G):(g + 1) * (H // G)]  # 8 local heads
        cm = (hsc ** -2.0).reshape(PAIRS, 2).T  # [2, PAIRS], n^(-2*sig)
        xa = np.asarray(x[bi], np.float32)
        NCc = n // 512
        in_maps.append({
            "xt": np.ascontiguousarray(
                xa.reshape(NCc, 512, KT, P).transpose(3, 0, 2, 1))
                .astype(ml_dtypes.bfloat16),
            "wq": _warr_ft(Wq, sl), "wk": _warr_ft(Wk, sl), "wv": _warr(Wv, sl),
            "wo": np.ascontiguousarray(
                np.asarray(Wo, np.float32)[sl].reshape(PAIRS, P, F)
                .transpose(1, 0, 2).astype(ml_dtypes.bfloat16)),
            "bq": np.ascontiguousarray(np.asarray(bq, np.float32)[sl].reshape(PAIRS, P).T),
            "bk": np.ascontiguousarray(np.asarray(bk, np.float32)[sl].reshape(PAIRS, P).T),
            "bv": np.ascontiguousarray(np.asarray(bv, np.float32)[sl]).astype(ml_dtypes.bfloat16),
            "cmsq": np.ascontiguousarray(cm.astype(np.float32)),
            "cind": _IND,
            "cblk": _BLK,
            "cones": _ONES,
        })
    return in_maps


def kernel(x, Wq, bq, Wk, bk, Wv, bv, Wo, bo, m, _trace=False):
    x = np.asarray(x, np.float32)
    b, n, f = x.shape
    nc = get_nc(n)
    in_maps = make_in_maps(x, Wq, bq, Wk, bk, Wv, bv, Wo, bo, m)
    res = bass_utils.run_bass_kernel_spmd(nc, in_maps,
                                          core_ids=list(range(NCORES)),
                                          trace=_trace)
    outs = [r["out"] for r in res.results]
    y = np.empty((b, n, f), np.float32)
    for bi in range(b):
        y[bi] = outs[2 * bi] + outs[2 * bi + 1]
    y += np.asarray(bo, np.float32).reshape(1, 1, f)
    if _trace:
        kernel._last_results = res
    return y


if __name__ == "__main__":
    # build-only smoke test (no device)
    nc = bacc.Bacc("TRN2", target_bir_lowering=False, debug=False,
                   num_devices=NCORES)
    build_core_program(nc, n=int(sys.argv[1]) if len(sys.argv) > 1 else 2048)
    print("build OK")
